# revision 5
# baseline (speedup 1.0000x reference)
"""Causal self-attention (B=4, S=2048, E=1024, H=16) on 8 TRN2 NeuronCores.

Sharding: core c handles batch b = c//2 and heads h in [8*(c%2), 8*(c%2)+8).
Each core computes its 8 heads' attention plus the partial output projection
(Megatron row-split); the host sums the two partials per batch and adds b_proj.

Kernel math per core (all matmuls fp32r):
  xT = x_b^T                       (PE transpose via matmul with identity)
  V  = x_b @ Wv_slice (+ones col)  (natural [s,d] layout, 8 heads wide)
  qkvT = Wqk_slice^T @ x_b^T       ([cols, s]: Q^T and K^T slices per head)
  per head: S^T = K Q^T (k on partitions), exp (+causal mask, +pad bias),
            AV^T with ones-row -> unnormalized out^T and softmax sums,
            normalize via reciprocal + K=1 broadcast matmul
  outT_partial = sum_pairs Wp_pair^T @ stacked(out^T pair)   [E, s]
Host: out[b] = (outT_{2b} + outT_{2b+1})^T + b_proj
"""
import numpy as np
from contextlib import ExitStack

import concourse.bass as bass
import concourse.tile as tile
import concourse.mybir as mybir
from concourse import bass_utils
from concourse.masks import make_identity

B, S, E, H = 4, 2048, 1024, 16
D = E // H              # 64
NCORES = 8
HPC = 8                 # heads per core
NPAIR = 4               # head pairs per core
CH = 512                # q chunk
NCHUNK = S // CH        # 4
KT = 128                # k tile
NKT = S // KT           # 16
ET = 128                # E tile
NET = E // ET           # 8
ST = 128                # s tile
NST = S // ST           # 16
NEG = -240000.0         # additive mask (pre-scale); *0.125 = -30000

F32 = mybir.dt.float32
F32R = mybir.dt.float32r


def _split_multi_waits(nc, max_waits=1):
    """This walrus build supports at most one sync wait per ISA instruction.
    Hoist extra waits onto same-engine NoOps inserted before the offender."""
    ctr = 0
    n_split = 0
    for f in nc.m.functions:
        for bb in f.blocks:
            insts = list(bb.instructions)
            out = []
            changed = False
            for ins in insts:
                si = getattr(ins, "sync_info", None)
                waits = list(si.on_wait) if (si and si.on_wait) else []
                if len(waits) > max_waits:
                    for w in waits[:-max_waits]:
                        ctr += 1
                        nop = mybir.InstNoOp(
                            name=f"I-wsplit-{ctr}", ins=[], outs=[],
                            engine=ins.engine)
                        nop.sync_info = mybir.SyncInfo(on_wait=[w], on_update=[])
                        out.append(nop)
                        n_split += 1
                    ins.sync_info = mybir.SyncInfo(
                        on_wait=waits[-max_waits:],
                        on_update=list(si.on_update or []))
                    changed = True
                out.append(ins)
            if changed:
                bb.instructions = out
    return n_split


def _build():
    nc = bass.Bass(trn_type="TRN2", target_bir_lowering=False, debug=False,
                   num_devices=NCORES)
    x = nc.dram_tensor("x", [S, E], F32R, kind="ExternalInput").ap()
    wqk = nc.dram_tensor("wqk", [E, 2 * HPC * D], F32R, kind="ExternalInput").ap()
    wv = nc.dram_tensor("wv", [E, HPC * D], F32R, kind="ExternalInput").ap()
    wp = nc.dram_tensor("wp", [HPC * D, E], F32R, kind="ExternalInput").ap()
    bqk = nc.dram_tensor("bqk", [128, 8], F32, kind="ExternalInput").ap()
    bv = nc.dram_tensor("bv", [1, HPC * D], F32R, kind="ExternalInput").ap()
    padb = nc.dram_tensor("padb", [128, NKT], F32, kind="ExternalInput").ap()
    outT = nc.dram_tensor("outT", [E, S], F32, kind="ExternalOutput").ap()

    with tile.TileContext(nc) as tc, ExitStack() as ctx:
        # ---------- long-lived pools ----------
        setup = ctx.enter_context(tc.tile_pool(name="setup", bufs=1))
        small_p = ctx.enter_context(tc.tile_pool(name="small", bufs=4))
        bcast_p = ctx.enter_context(tc.tile_pool(name="bcast", bufs=2))
        hb_p = ctx.enter_context(tc.tile_pool(name="hbst", bufs=2))
        outacc_p = ctx.enter_context(tc.tile_pool(name="outacc", bufs=1))
        vaug_p = ctx.enter_context(tc.tile_pool(name="vaug", bufs=1))
        psum_proj = ctx.enter_context(
            tc.tile_pool(name="ps_proj", bufs=2, space="PSUM"))

        # ---------- setup constants ----------
        identf = setup.tile([128, 128], F32)
        make_identity(nc, identf[:])
        ident = setup.tile([128, 128], F32R)
        nc.vector.tensor_copy(ident[:], identf[:])

        # causal additive strip M[p, u] = 0 if u - p >= 384 else NEG
        # diag tile offset o in {0,128,256,384}: mask = M[:, 384-o : 896-o]
        mstrip = setup.tile([128, 896], F32)
        nc.gpsimd.memset(mstrip[:], 0.0)
        nc.gpsimd.affine_select(
            out=mstrip[:], in_=mstrip[:],
            compare_op=mybir.AluOpType.is_ge, fill=NEG,
            base=-384, channel_multiplier=-1, pattern=[[1, 896]])

        ones_f32 = setup.tile([1, 128], F32)
        nc.gpsimd.memset(ones_f32[:], 1.0)
        ones64 = setup.tile([1, 64], F32R)
        nc.vector.tensor_copy(ones64[:], ones_f32[:, 0:64])
        ones128 = setup.tile([1, 128], F32R)
        nc.vector.tensor_copy(ones128[:], ones_f32[:])
        ones8 = setup.tile([128, 8], F32)
        nc.gpsimd.memset(ones8[:], 1.0)

        padb_sb = setup.tile([128, NKT], F32)
        nc.sync.dma_start(out=padb_sb[:], in_=padb)
        bqk_sb = setup.tile([128, 8], F32)
        nc.sync.dma_start(out=bqk_sb[:], in_=bqk)
        bv_sb = setup.tile([1, HPC * D], F32R)
        nc.sync.dma_start(out=bv_sb[:], in_=bv)

        # ---------- persistent data tiles ----------
        vaug = vaug_p.tile([128, NST, HPC, 68], F32R)
        outacc = outacc_p.tile([128, NPAIR, NCHUNK, CH], F32R)
        for st in range(NST):
            nc.vector.tensor_copy(vaug[:, st, :, 64:65],
                                  ones8[:].unsqueeze(2))

        with ExitStack() as xts:
            xT_p = xts.enter_context(tc.tile_pool(name="xT", bufs=1))
            xT = xT_p.tile([128, NET, S], F32R)

            # ---------- phase A: transpose x, V proj ----------
            with ExitStack() as pa:
                xnat_p = pa.enter_context(tc.tile_pool(name="xnat", bufs=2))
                wv_p = pa.enter_context(tc.tile_pool(name="wv", bufs=1))
                psum_tr = pa.enter_context(
                    tc.tile_pool(name="ps_tr", bufs=2, space="PSUM"))

                wvt = wv_p.tile([128, NET, HPC * D], F32R)
                for e in range(NET):
                    nc.sync.dma_start(out=wvt[:, e, :],
                                      in_=wv[e * ET:(e + 1) * ET, :])

                # A1: x -> xT
                for st in range(NST):
                    xt = xnat_p.tile([128, E], F32R, tag="xn")
                    nc.sync.dma_start(out=xt[:], in_=x[st * ST:(st + 1) * ST, :])
                    for e in range(NET):
                        pt = psum_tr.tile([128, 128], F32, tag="tr")
                        nc.tensor.matmul(pt[:], xt[:, e * ET:(e + 1) * ET],
                                         ident[:], start=True, stop=True)
                        nc.vector.tensor_copy(
                            xT[:, e, st * ST:(st + 1) * ST], pt[:])

                # A2: V = x @ Wv (+bias via K=1 ones matmul), + ones col
                for st in range(NST):
                    pv = psum_proj.tile([128, HPC * D], F32, tag="pj")
                    for e in range(NET):
                        nc.tensor.matmul(
                            pv[:], xT[:, e, st * ST:(st + 1) * ST],
                            wvt[:, e, :], start=(e == 0), stop=False)
                    nc.tensor.matmul(pv[:], ones128[:], bv_sb[:],
                                     start=False, stop=True)
                    nc.vector.tensor_copy(
                        vaug[:, st, :, 0:64],
                        pv[:].rearrange("p (h d) -> p h d", h=HPC))

            # ---------- pairs: QK proj + attention ----------
            with ExitStack() as pp:
                qkvT_p = pp.enter_context(tc.tile_pool(name="qkvT", bufs=2))
                wqk_p = pp.enter_context(tc.tile_pool(name="wqks", bufs=4))
                attn_p = pp.enter_context(tc.tile_pool(name="attnT", bufs=4))
                psum_S = pp.enter_context(
                    tc.tile_pool(name="ps_S", bufs=3, space="PSUM"))
                psum_av = pp.enter_context(
                    tc.tile_pool(name="ps_av", bufs=2, space="PSUM"))
                psum_b = pp.enter_context(
                    tc.tile_pool(name="ps_b", bufs=1, space="PSUM"))

                for p in range(NPAIR):
                    # QK proj for this pair: qkvT[:, 0, :] = Q^T, [:, 1, :] = K^T
                    qkvT = qkvT_p.tile([128, 2, S], F32R, tag="qkvT")
                    for ct in range(2):
                        for j in range(NCHUNK):
                            pq = psum_proj.tile([128, CH], F32, tag="pj")
                            for e in range(NET):
                                wt = wqk_p.tile([128, 128], F32R, tag="wqk")
                                nc.sync.dma_start(
                                    out=wt[:],
                                    in_=wqk[e * ET:(e + 1) * ET,
                                            ct * 512 + p * 128:
                                            ct * 512 + (p + 1) * 128])
                                nc.tensor.matmul(
                                    pq[:], wt[:],
                                    xT[:, e, j * CH:(j + 1) * CH],
                                    start=(e == 0), stop=(e == NET - 1))
                            nc.scalar.activation(
                                out=qkvT[:, ct, j * CH:(j + 1) * CH],
                                in_=pq[:],
                                func=mybir.ActivationFunctionType.Identity,
                                bias=bqk_sb[:, ct * 4 + p:ct * 4 + p + 1],
                                scale=1.0)

                    # attention for this pair
                    for j in range(NCHUNK):
                        pav = {}
                        for hh in range(2):
                            pav[hh] = psum_av.tile([65, CH], F32, tag="av",
                                                   name="pav")
                        nkt = 4 * (j + 1)       # causal: k tiles 0..4j+3
                        for i in range(nkt):
                            for hh in range(2):
                                lo, hi = (0, 64) if hh == 0 else (64, 128)
                                ps = psum_S.tile([128, CH], F32, tag="S")
                                nc.tensor.matmul(
                                    ps[:],
                                    qkvT[lo:hi, 1, i * KT:(i + 1) * KT],
                                    qkvT[lo:hi, 0, j * CH:(j + 1) * CH],
                                    start=True, stop=True)
                                if i >= 4 * j:  # diagonal-crossing tile
                                    o = 128 * i - 512 * j
                                    nc.vector.tensor_add(
                                        ps[:], ps[:],
                                        mstrip[:, 384 - o:896 - o])
                                at = attn_p.tile([128, CH], F32R, tag="at")
                                nc.scalar.activation(
                                    out=at[:], in_=ps[:],
                                    func=mybir.ActivationFunctionType.Exp,
                                    bias=padb_sb[:, i:i + 1], scale=0.125)
                                nc.tensor.matmul(
                                    pav[hh][:],
                                    vaug[:, i, 2 * p + hh, 0:65], at[:],
                                    start=(i == 0), stop=(i == nkt - 1))
                        # normalize + stack the pair
                        for hh in range(2):
                            rec = small_p.tile([1, CH], F32R, tag="rec")
                            with nc.allow_low_precision(
                                    reason="softmax recip to f32r"):
                                nc.vector.reciprocal(rec[:], pav[hh][64:65, :])
                            pb = psum_b.tile([64, CH], F32, tag="bc")
                            nc.tensor.matmul(pb[:], ones64[:], rec[:],
                                             start=True, stop=True)
                            bc = bcast_p.tile([64, CH], F32R, tag="bc2")
                            nc.vector.tensor_copy(bc[:], pb[:])
                            if hh == 0:
                                nc.vector.tensor_mul(
                                    outacc[0:64, p, j, :],
                                    pav[hh][0:64, :], bc[:])
                            else:
                                hb = hb_p.tile([64, CH], F32R, tag="hb")
                                nc.vector.tensor_mul(hb[:], pav[hh][0:64, :],
                                                     bc[:])
                                nc.sync.dma_start(
                                    out=outacc[64:128, p, j, :], in_=hb[:])

        # ---------- output projection ----------
        with ExitStack() as po_ctx:
            wp_p = po_ctx.enter_context(tc.tile_pool(name="wp", bufs=1))
            ostage_p = po_ctx.enter_context(tc.tile_pool(name="ostage", bufs=3))
            wpt = wp_p.tile([128, NPAIR, E], F32R)
            for p in range(NPAIR):
                nc.sync.dma_start(out=wpt[:, p, :],
                                  in_=wp[p * 128:(p + 1) * 128, :])
            for e in range(NET):
                for j in range(NCHUNK):
                    po = psum_proj.tile([128, CH], F32, tag="pj")
                    for p in range(NPAIR):
                        nc.tensor.matmul(
                            po[:], wpt[:, p, e * ET:(e + 1) * ET],
                            outacc[:, p, j, :],
                            start=(p == 0), stop=(p == NPAIR - 1))
                    os = ostage_p.tile([128, CH], F32, tag="os")
                    nc.vector.tensor_copy(os[:], po[:])
                    nc.sync.dma_start(
                        out=outT[e * ET:(e + 1) * ET, j * CH:(j + 1) * CH],
                        in_=os[:])

    _split_multi_waits(nc)
    return nc


_NC = None


def _get_nc():
    global _NC
    if _NC is None:
        _NC = _build()
    return _NC


def kernel(x, attention_mask, W_qkv, b_qkv, W_proj, b_proj):
    x = np.asarray(x, dtype=np.float32)
    attention_mask = np.asarray(attention_mask)
    W_qkv = np.ascontiguousarray(np.asarray(W_qkv, dtype=np.float32))
    b_qkv = np.asarray(b_qkv, dtype=np.float32)
    W_proj = np.ascontiguousarray(np.asarray(W_proj, dtype=np.float32))
    b_proj = np.asarray(b_proj, dtype=np.float32)

    in_maps = []
    for c in range(NCORES):
        b = c // 2
        h0 = (c % 2) * HPC
        cols = slice(h0 * D, (h0 + HPC) * D)          # within one of q/k/v blocks
        wq = W_qkv[:, 0 * E + h0 * D:0 * E + (h0 + HPC) * D]
        wk = W_qkv[:, 1 * E + h0 * D:1 * E + (h0 + HPC) * D]
        wv = W_qkv[:, 2 * E + h0 * D:2 * E + (h0 + HPC) * D]
        bq = b_qkv[0 * E + h0 * D:0 * E + (h0 + HPC) * D]
        bk = b_qkv[1 * E + h0 * D:1 * E + (h0 + HPC) * D]
        bvv = b_qkv[2 * E + h0 * D:2 * E + (h0 + HPC) * D]
        wqk = np.ascontiguousarray(np.concatenate([wq, wk], axis=1))
        bqk = np.ascontiguousarray(
            np.concatenate([bq, bk]).reshape(8, 128).T)   # [128, 8] per col-tile
        padrow = np.where(attention_mask[b] != 0, 0.0, -30000.0).astype(np.float32)
        padb = np.ascontiguousarray(padrow.reshape(NKT, 128).T)  # [128, NKT]
        in_maps.append({
            "x": np.ascontiguousarray(x[b]),
            "wqk": wqk,
            "wv": np.ascontiguousarray(wv),
            "wp": np.ascontiguousarray(W_proj[h0 * D:(h0 + HPC) * D, :]),
            "bqk": bqk,
            "bv": np.ascontiguousarray(bvv.reshape(1, HPC * D)),
            "padb": padb,
        })

    nc = _get_nc()
    res = bass_utils.run_bass_kernel_spmd(nc, in_maps, core_ids=list(range(NCORES)))

    out = np.empty((B, S, E), dtype=np.float32)
    for b in range(B):
        acc = res.results[2 * b]["outT"] + res.results[2 * b + 1]["outT"]
        out[b] = acc.T + b_proj[None, :]
    return out


# revision 7
# speedup vs baseline: 1.0164x; 1.0164x over previous
"""Causal self-attention (B=4, S=2048, E=1024, H=16) on 8 TRN2 NeuronCores.

Sharding: core c handles batch b = c//2 and heads h in [8*(c%2), 8*(c%2)+8).
Each core computes its 8 heads' attention plus the partial output projection
(Megatron row-split); the host sums the two partials per batch and adds b_proj.

Kernel math per core (all matmuls fp32r):
  xT = x_b^T                       (PE transpose via matmul with identity)
  V  = x_b @ Wv_slice (+ones col)  (natural [s,d] layout, 8 heads wide)
  qkvT = Wqk_slice^T @ x_b^T       ([cols, s]: Q^T and K^T slices per head)
  per head: S^T = K Q^T (k on partitions), exp (+causal mask, +pad bias),
            AV^T with ones-row -> unnormalized out^T and softmax sums,
            normalize via reciprocal + K=1 broadcast matmul
  outT_partial = sum_pairs Wp_pair^T @ stacked(out^T pair)   [E, s]
Host: out[b] = (outT_{2b} + outT_{2b+1})^T + b_proj
"""
import numpy as np
from contextlib import ExitStack

import concourse.bass as bass
import concourse.tile as tile
import concourse.mybir as mybir
from concourse import bass_utils
from concourse.masks import make_identity

B, S, E, H = 4, 2048, 1024, 16
D = E // H              # 64
NCORES = 8
HPC = 8                 # heads per core
NPAIR = 4               # head pairs per core
CH = 512                # q chunk
NCHUNK = S // CH        # 4
KT = 128                # k tile
NKT = S // KT           # 16
ET = 128                # E tile
NET = E // ET           # 8
ST = 128                # s tile
NST = S // ST           # 16
NEG = -240000.0         # additive mask (pre-scale); *0.125 = -30000

F32 = mybir.dt.float32
F32R = mybir.dt.float32r


def _split_multi_waits(nc, max_waits=1):
    """This walrus build supports at most one sync wait per ISA instruction.
    Hoist extra waits onto same-engine NoOps inserted before the offender."""
    ctr = 0
    n_split = 0
    for f in nc.m.functions:
        for bb in f.blocks:
            insts = list(bb.instructions)
            out = []
            changed = False
            for ins in insts:
                si = getattr(ins, "sync_info", None)
                waits = list(si.on_wait) if (si and si.on_wait) else []
                if len(waits) > max_waits:
                    for w in waits[:-max_waits]:
                        ctr += 1
                        nop = mybir.InstNoOp(
                            name=f"I-wsplit-{ctr}", ins=[], outs=[],
                            engine=ins.engine)
                        nop.sync_info = mybir.SyncInfo(on_wait=[w], on_update=[])
                        out.append(nop)
                        n_split += 1
                    ins.sync_info = mybir.SyncInfo(
                        on_wait=waits[-max_waits:],
                        on_update=list(si.on_update or []))
                    changed = True
                out.append(ins)
            if changed:
                bb.instructions = out
    return n_split


def _build():
    nc = bass.Bass(trn_type="TRN2", target_bir_lowering=False, debug=False,
                   num_devices=NCORES)
    x = nc.dram_tensor("x", [S, E], F32R, kind="ExternalInput").ap()
    wqk = nc.dram_tensor("wqk", [E, 2 * HPC * D], F32R, kind="ExternalInput").ap()
    wv = nc.dram_tensor("wv", [E, HPC * D], F32R, kind="ExternalInput").ap()
    wp = nc.dram_tensor("wp", [HPC * D, E], F32R, kind="ExternalInput").ap()
    bqk = nc.dram_tensor("bqk", [128, 8], F32, kind="ExternalInput").ap()
    bv = nc.dram_tensor("bv", [1, HPC * D], F32R, kind="ExternalInput").ap()
    padb = nc.dram_tensor("padb", [128, NKT], F32, kind="ExternalInput").ap()
    outT = nc.dram_tensor("outT", [E, S], F32, kind="ExternalOutput").ap()

    with tile.TileContext(nc) as tc, ExitStack() as ctx:
        # ---------- long-lived pools ----------
        setup = ctx.enter_context(tc.tile_pool(name="setup", bufs=1))
        small_p = ctx.enter_context(tc.tile_pool(name="small", bufs=4))
        bcast_p = ctx.enter_context(tc.tile_pool(name="bcast", bufs=2))
        hb_p = ctx.enter_context(tc.tile_pool(name="hbst", bufs=2))
        outacc_p = ctx.enter_context(tc.tile_pool(name="outacc", bufs=1))
        vaug_p = ctx.enter_context(tc.tile_pool(name="vaug", bufs=1))
        psum_proj = ctx.enter_context(
            tc.tile_pool(name="ps_proj", bufs=2, space="PSUM"))

        # ---------- setup constants ----------
        identf = setup.tile([128, 128], F32)
        make_identity(nc, identf[:])
        ident = setup.tile([128, 128], F32R)
        nc.vector.tensor_copy(ident[:], identf[:])

        # causal additive triangle: tri128[k, c] = 0 if c >= k else NEG
        tri128 = setup.tile([128, 128], F32)
        nc.gpsimd.memset(tri128[:], 0.0)
        nc.gpsimd.affine_select(
            out=tri128[:], in_=tri128[:],
            compare_op=mybir.AluOpType.is_ge, fill=NEG,
            base=0, channel_multiplier=-1, pattern=[[1, 128]])

        ones_f32 = setup.tile([1, 128], F32)
        nc.gpsimd.memset(ones_f32[:], 1.0)
        ones64 = setup.tile([1, 64], F32R)
        nc.vector.tensor_copy(ones64[:], ones_f32[:, 0:64])
        ones128 = setup.tile([1, 128], F32R)
        nc.vector.tensor_copy(ones128[:], ones_f32[:])
        ones8 = setup.tile([128, 8], F32)
        nc.gpsimd.memset(ones8[:], 1.0)

        padb_sb = setup.tile([128, NKT], F32)
        nc.sync.dma_start(out=padb_sb[:], in_=padb)
        bqk_sb = setup.tile([128, 8], F32)
        nc.sync.dma_start(out=bqk_sb[:], in_=bqk)
        bv_sb = setup.tile([1, HPC * D], F32R)
        nc.sync.dma_start(out=bv_sb[:], in_=bv)

        # ---------- persistent data tiles ----------
        vaug = vaug_p.tile([128, NST, HPC, 68], F32R)
        outacc = outacc_p.tile([128, NPAIR, NCHUNK, CH], F32R)
        for st in range(NST):
            nc.vector.tensor_copy(vaug[:, st, :, 64:65],
                                  ones8[:].unsqueeze(2))

        with ExitStack() as xts:
            xT_p = xts.enter_context(tc.tile_pool(name="xT", bufs=1))
            xT = xT_p.tile([128, NET, S], F32R)

            # ---------- phase A: transpose x, V proj ----------
            with ExitStack() as pa:
                xnat_p = pa.enter_context(tc.tile_pool(name="xnat", bufs=5))
                wv_p = pa.enter_context(tc.tile_pool(name="wv", bufs=1))
                psum_tr = pa.enter_context(
                    tc.tile_pool(name="ps_tr", bufs=2, space="PSUM"))

                wvt = wv_p.tile([128, NET, HPC * D], F32R)
                for e in range(NET):
                    nc.sync.dma_start(out=wvt[:, e, :],
                                      in_=wv[e * ET:(e + 1) * ET, :])

                # A1: x -> xT (is_transpose, 4 s-tiles batched per psum bank)
                for stg in range(NST // 4):
                    xts_ = []
                    for k in range(4):
                        st = stg * 4 + k
                        xt = xnat_p.tile([128, E], F32R, tag="xn", name="xt")
                        nc.sync.dma_start(out=xt[:],
                                          in_=x[st * ST:(st + 1) * ST, :])
                        xts_.append(xt)
                    for e in range(NET):
                        pt = psum_tr.tile([128, 512], F32R, tag="tr")
                        for k in range(4):
                            nc.tensor.matmul(
                                pt[:, k * 128:(k + 1) * 128],
                                xts_[k][:, e * ET:(e + 1) * ET],
                                ident[:], is_transpose=True,
                                start=True, stop=True)
                        nc.vector.tensor_copy(
                            xT[:, e, stg * 512:(stg + 1) * 512], pt[:])

                # A2: V = x @ Wv (+bias via K=1 ones matmul), + ones col
                for st in range(NST):
                    pv = psum_proj.tile([128, HPC * D], F32, tag="pj")
                    for e in range(NET):
                        nc.tensor.matmul(
                            pv[:], xT[:, e, st * ST:(st + 1) * ST],
                            wvt[:, e, :], start=(e == 0), stop=False)
                    nc.tensor.matmul(pv[:], ones128[:], bv_sb[:],
                                     start=False, stop=True)
                    nc.vector.tensor_copy(
                        vaug[:, st, :, 0:64],
                        pv[:].rearrange("p (h d) -> p h d", h=HPC))

            # ---------- pairs: QK proj + attention ----------
            with ExitStack() as pp:
                qkvT_p = pp.enter_context(tc.tile_pool(name="qkvT", bufs=2))
                wqk_p = pp.enter_context(tc.tile_pool(name="wqks", bufs=4))
                attn_p = pp.enter_context(tc.tile_pool(name="attnT", bufs=4))
                psum_S = pp.enter_context(
                    tc.tile_pool(name="ps_S", bufs=3, space="PSUM"))
                psum_av = pp.enter_context(
                    tc.tile_pool(name="ps_av", bufs=2, space="PSUM"))
                psum_b = pp.enter_context(
                    tc.tile_pool(name="ps_b", bufs=1, space="PSUM"))

                for p in range(NPAIR):
                    # QK proj for this pair: qkvT[:, 0, :] = Q^T, [:, 1, :] = K^T
                    qkvT = qkvT_p.tile([128, 2, S], F32R, tag="qkvT")
                    for ct in range(2):
                        for j in range(NCHUNK):
                            pq = psum_proj.tile([128, CH], F32, tag="pj")
                            for e in range(NET):
                                wt = wqk_p.tile([128, 128], F32R, tag="wqk")
                                nc.sync.dma_start(
                                    out=wt[:],
                                    in_=wqk[e * ET:(e + 1) * ET,
                                            ct * 512 + p * 128:
                                            ct * 512 + (p + 1) * 128])
                                nc.tensor.matmul(
                                    pq[:], wt[:],
                                    xT[:, e, j * CH:(j + 1) * CH],
                                    start=(e == 0), stop=(e == NET - 1))
                            nc.vector.tensor_scalar_add(
                                out=qkvT[:, ct, j * CH:(j + 1) * CH],
                                in0=pq[:],
                                scalar1=bqk_sb[:, ct * 4 + p:ct * 4 + p + 1])

                    # attention for this pair
                    for j in range(NCHUNK):
                        pav = {}
                        for hh in range(2):
                            pav[hh] = psum_av.tile([65, CH], F32, tag="av",
                                                   name="pav")
                        nkt = 4 * (j + 1)       # causal: k tiles 0..4j+3
                        for i in range(nkt):
                            for hh in range(2):
                                lo, hi = (0, 64) if hh == 0 else (64, 128)
                                ps = psum_S.tile([128, CH], F32, tag="S")
                                nc.tensor.matmul(
                                    ps[:],
                                    qkvT[lo:hi, 1, i * KT:(i + 1) * KT],
                                    qkvT[lo:hi, 0, j * CH:(j + 1) * CH],
                                    start=True, stop=True)
                                at = attn_p.tile([128, CH], F32R, tag="at")
                                if i >= 4 * j:  # diagonal-crossing tile
                                    o = 128 * i - 512 * j
                                    if o > 0:
                                        nc.vector.tensor_scalar_mul(
                                            out=at[:, 0:o], in0=ps[:, 0:o],
                                            scalar1=0.0)
                                    nc.vector.tensor_add(
                                        ps[:, o:o + 128], ps[:, o:o + 128],
                                        tri128[:])
                                    nc.scalar.activation(
                                        out=at[:, o:CH], in_=ps[:, o:CH],
                                        func=mybir.ActivationFunctionType.Exp,
                                        bias=padb_sb[:, i:i + 1], scale=0.125)
                                else:
                                    nc.scalar.activation(
                                        out=at[:], in_=ps[:],
                                        func=mybir.ActivationFunctionType.Exp,
                                        bias=padb_sb[:, i:i + 1], scale=0.125)
                                nc.tensor.matmul(
                                    pav[hh][:],
                                    vaug[:, i, 2 * p + hh, 0:65], at[:],
                                    start=(i == 0), stop=(i == nkt - 1))
                        # normalize + stack the pair
                        for hh in range(2):
                            rec = small_p.tile([1, CH], F32R, tag="rec")
                            with nc.allow_low_precision(
                                    reason="softmax recip to f32r"):
                                nc.vector.reciprocal(rec[:], pav[hh][64:65, :])
                            pb = psum_b.tile([64, CH], F32, tag="bc")
                            nc.tensor.matmul(pb[:], ones64[:], rec[:],
                                             start=True, stop=True)
                            bc = bcast_p.tile([64, CH], F32R, tag="bc2")
                            nc.vector.tensor_copy(bc[:], pb[:])
                            if hh == 0:
                                nc.vector.tensor_mul(
                                    outacc[0:64, p, j, :],
                                    pav[hh][0:64, :], bc[:])
                            else:
                                hb = hb_p.tile([64, CH], F32R, tag="hb")
                                nc.vector.tensor_mul(hb[:], pav[hh][0:64, :],
                                                     bc[:])
                                nc.sync.dma_start(
                                    out=outacc[64:128, p, j, :], in_=hb[:])

        # ---------- output projection ----------
        with ExitStack() as po_ctx:
            wp_p = po_ctx.enter_context(tc.tile_pool(name="wp", bufs=1))
            ostage_p = po_ctx.enter_context(tc.tile_pool(name="ostage", bufs=3))
            wpt = wp_p.tile([128, NPAIR, E], F32R)
            for p in range(NPAIR):
                nc.sync.dma_start(out=wpt[:, p, :],
                                  in_=wp[p * 128:(p + 1) * 128, :])
            for e in range(NET):
                for j in range(NCHUNK):
                    po = psum_proj.tile([128, CH], F32, tag="pj")
                    for p in range(NPAIR):
                        nc.tensor.matmul(
                            po[:], wpt[:, p, e * ET:(e + 1) * ET],
                            outacc[:, p, j, :],
                            start=(p == 0), stop=(p == NPAIR - 1))
                    os = ostage_p.tile([128, CH], F32, tag="os")
                    nc.vector.tensor_copy(os[:], po[:])
                    nc.sync.dma_start(
                        out=outT[e * ET:(e + 1) * ET, j * CH:(j + 1) * CH],
                        in_=os[:])

    _split_multi_waits(nc)
    return nc


_NC = None


def _get_nc():
    global _NC
    if _NC is None:
        _NC = _build()
    return _NC


def kernel(x, attention_mask, W_qkv, b_qkv, W_proj, b_proj):
    x = np.asarray(x, dtype=np.float32)
    attention_mask = np.asarray(attention_mask)
    W_qkv = np.ascontiguousarray(np.asarray(W_qkv, dtype=np.float32))
    b_qkv = np.asarray(b_qkv, dtype=np.float32)
    W_proj = np.ascontiguousarray(np.asarray(W_proj, dtype=np.float32))
    b_proj = np.asarray(b_proj, dtype=np.float32)

    in_maps = []
    for c in range(NCORES):
        b = c // 2
        h0 = (c % 2) * HPC
        cols = slice(h0 * D, (h0 + HPC) * D)          # within one of q/k/v blocks
        wq = W_qkv[:, 0 * E + h0 * D:0 * E + (h0 + HPC) * D]
        wk = W_qkv[:, 1 * E + h0 * D:1 * E + (h0 + HPC) * D]
        wv = W_qkv[:, 2 * E + h0 * D:2 * E + (h0 + HPC) * D]
        bq = b_qkv[0 * E + h0 * D:0 * E + (h0 + HPC) * D]
        bk = b_qkv[1 * E + h0 * D:1 * E + (h0 + HPC) * D]
        bvv = b_qkv[2 * E + h0 * D:2 * E + (h0 + HPC) * D]
        wqk = np.ascontiguousarray(np.concatenate([wq, wk], axis=1))
        bqk = np.ascontiguousarray(
            np.concatenate([bq, bk]).reshape(8, 128).T)   # [128, 8] per col-tile
        padrow = np.where(attention_mask[b] != 0, 0.0, -30000.0).astype(np.float32)
        padb = np.ascontiguousarray(padrow.reshape(NKT, 128).T)  # [128, NKT]
        in_maps.append({
            "x": np.ascontiguousarray(x[b]),
            "wqk": wqk,
            "wv": np.ascontiguousarray(wv),
            "wp": np.ascontiguousarray(W_proj[h0 * D:(h0 + HPC) * D, :]),
            "bqk": bqk,
            "bv": np.ascontiguousarray(bvv.reshape(1, HPC * D)),
            "padb": padb,
        })

    nc = _get_nc()
    res = bass_utils.run_bass_kernel_spmd(nc, in_maps, core_ids=list(range(NCORES)))

    out = np.empty((B, S, E), dtype=np.float32)
    for b in range(B):
        acc = res.results[2 * b]["outT"] + res.results[2 * b + 1]["outT"]
        out[b] = acc.T + b_proj[None, :]
    return out
